# revision 3
# baseline (speedup 1.0000x reference)
"""Coattentive layer kernel for Trainium2, data-parallel over batch on 8 cores.

Per batch (Lc=2048, Lq=256, d=512; sentinel-prepended => c:2049 (pad 2176),
q:257 (pad 384)):

  QpT[e,q]   = tanh(W^T-projection of question^T + b)          (fp32r matmuls)
  aff[c,q]   = C @ Qp^T  via CT (PE-transposed context)        (fp32r matmuls)
  E0[c,q]    = exp(aff - 45) stored bf16; E0T = PE-transpose of E0.
  Masked-softmax normalizers are N=1 ones-column matmuls sharing the value
  matmuls' stationary weights; pad/key masks are folded multiplicatively into
  the bf16 casts of the value-side operands (Cmask, Qp, S_c, S_q).
  S_c  = (1/Z_c) E_c^T C           -> out_q[:, 512:]
  S_q  = (1/Z_q) E_q^T Qp          -> out_c[:, 512:]
  CC   = (1/Z_q) E_q^T S_c         -> out_c[:, :512]
  CQ   = (1/Z_c) E_c^T S_q         -> out_q[:, :512]
"""

import numpy as np

B, LC, LQ, D = 16, 2048, 256, 512
NCORES = 8
BLOC = B // NCORES
CP = LC + 1       # 2049
QP = LQ + 1       # 257
QPE = QP + 1      # 258: fp32r matmuls need an even moving free dim
CTN = 17          # c tiles (2049 -> 2176)
QTN = 3           # q tiles (257 -> 384)
CPAD = CTN * 128  # 2176
QPAD = QTN * 128  # 384
DK = 4            # 512 / 128
NEG_G = -45.0     # global exp shift (affinity range is ~[-84, 86])
E0_PAD = 1e-20    # keeps Z > 0 for padding rows (avoids 1/0 -> NaN)

_CACHE = {}


def _build(repeat=1):
    import contextlib
    import concourse.bacc as bacc
    import concourse.tile as tile
    from concourse import mybir
    from concourse.masks import make_identity

    f32 = mybir.dt.float32
    f32r = mybir.dt.float32r
    bf16 = mybir.dt.bfloat16
    AF = mybir.ActivationFunctionType

    nc = bacc.Bacc("TRN2", target_bir_lowering=False, debug=False,
                   num_devices=NCORES)

    ctx_in = nc.declare_dram_parameter("context", [BLOC, LC, D], f32, False)
    q0t_in = nc.declare_dram_parameter("q0t", [BLOC, D, QPAD], f32, False)
    sent_in = nc.declare_dram_parameter("sent", [2, D], f32, False)
    wt_in = nc.declare_dram_parameter("proj_wt", [D, D], f32, False)
    b_in = nc.declare_dram_parameter("b_pt", [128, DK], f32, False)
    cm_in = nc.declare_dram_parameter("cm01", [BLOC, 128, CTN], f32, False)
    qm_in = nc.declare_dram_parameter("qm01", [BLOC, 128, QTN], f32, False)
    outc = nc.declare_dram_parameter("out_c", [BLOC, LC, 2 * D], f32, True)
    outq = nc.declare_dram_parameter("out_q", [BLOC, LQ, 2 * D], f32, True)

    with tile.TileContext(nc) as tc:
        with (
            tc.tile_pool(name="consts", bufs=1) as consts,
            tc.tile_pool(name="big", bufs=1) as big,
            tc.tile_pool(name="stage", bufs=3) as stage,
            tc.tile_pool(name="ps_mm", bufs=3, space="PSUM") as ps_mm,
            tc.tile_pool(name="ps_z", bufs=2, space="PSUM") as ps_z,
            tc.tile_pool(name="ps_tr", bufs=2, space="PSUM") as ps_tr,
        ):
            ident = consts.tile([128, 128], f32, tag="ident")
            make_identity(nc, ident)
            ident_b = consts.tile([128, 128], bf16, tag="identb")
            nc.vector.tensor_copy(ident_b, ident)
            neg_g = consts.tile([128, 1], f32, tag="negg")
            nc.vector.memset(neg_g, NEG_G)

            wt_f = consts.tile([128, DK, D], f32, tag="wtf")
            nc.sync.dma_start(
                out=wt_f, in_=wt_in[:].rearrange("(k p) e -> p k e", p=128))
            wt_r = consts.tile([128, DK, D], f32r, tag="wtr")
            nc.vector.tensor_copy(wt_r, wt_f)
            b_sb = consts.tile([128, DK], f32, tag="bsb")
            nc.sync.dma_start(out=b_sb, in_=b_in[:])

            # alternating psum-eviction engine (balances ACT vs DVE)
            ev_state = [0]

            def evict(dst, src, scale=None):
                ev_state[0] ^= 1
                if ev_state[0]:
                    nc.scalar.activation(
                        dst, src, AF.Copy,
                        **({"scale": scale} if scale is not None else {}))
                elif scale is not None:
                    nc.vector.tensor_scalar_mul(dst, src, scale)
                else:
                    nc.vector.tensor_copy(dst, src)

            rep_ctx = (tc.For_i(0, repeat, 1,
                                hint_engines=(mybir.EngineType.PE,
                                              mybir.EngineType.Activation,
                                              mybir.EngineType.DVE,
                                              mybir.EngineType.SP))
                       if repeat > 1 else contextlib.nullcontext())
            with rep_ctx:
              for b in range(BLOC):
                # ---------- loads ----------
                cnat = big.tile([128, CTN, D], f32, tag="cnat")
                nc.vector.memset(cnat[:, CTN - 1, :], 0.0)
                nc.sync.dma_start(out=cnat[0:1, 0, :], in_=sent_in[0:1, :])
                nc.sync.dma_start(out=cnat[1:128, 0, :], in_=ctx_in[b, 0:127, :])
                for t in range(1, CTN - 1):
                    nc.sync.dma_start(
                        out=cnat[:, t, :],
                        in_=ctx_in[b, t * 128 - 1:t * 128 + 127, :])
                nc.sync.dma_start(
                    out=cnat[0:1, CTN - 1, :], in_=ctx_in[b, LC - 1:LC, :])

                q0t = big.tile([128, DK, QPAD], f32r, tag="q0t")
                nc.sync.dma_start(
                    out=q0t,
                    in_=q0t_in[b].rearrange("(k p) q -> p k q", p=128).bitcast(f32r))

                cm = big.tile([128, CTN], f32, tag="cm")
                nc.sync.dma_start(out=cm, in_=cm_in[b])
                qm = big.tile([128, QTN], f32, tag="qm")
                nc.sync.dma_start(out=qm, in_=qm_in[b])
                cmb = big.tile([128, CTN], bf16, tag="cmb")
                nc.gpsimd.tensor_copy(cmb, cm)
                qmb = big.tile([128, QTN], bf16, tag="qmb")
                nc.gpsimd.tensor_copy(qmb, qm)

                # bf16 masked context for the value matmuls (on Pool engine)
                cmask = big.tile([128, CTN, D], bf16, tag="cmask")
                for t in range(CTN):
                    nc.gpsimd.tensor_scalar_mul(
                        cmask[:, t, :], cnat[:, t, :], cm[:, t:t + 1])

                # ---------- projection: QpT = tanh(W^T q + b) ----------
                qpt = big.tile([128, DK, QPAD], f32, tag="qpt")
                qpt_r = big.tile([128, DK, QPAD], f32r, tag="qptr")
                nc.gpsimd.memset(qpt[:, :, QPE:], 0.0)
                for ek in range(DK):
                    ps = ps_mm.tile([128, QPE], f32, tag="mm")
                    for dk in range(DK):
                        nc.tensor.matmul(
                            ps, wt_r[:, dk, ek * 128:(ek + 1) * 128],
                            q0t[:, dk, 0:QPE],
                            start=(dk == 0), stop=(dk == DK - 1))
                    nc.scalar.activation(
                        qpt[:, ek, 0:QPE], ps, AF.Tanh, bias=b_sb[:, ek:ek + 1])
                    nc.vector.tensor_copy(qpt_r[:, ek, 0:QPE], qpt[:, ek, 0:QPE])

                # ---------- Qp natural (bf16, q-mask folded) ----------
                qp_b = big.tile([128, QTN, D], bf16, tag="qp")
                for t in range(QTN):
                    for ek in range(DK):
                        ps = ps_tr.tile([128, 128], f32, tag="tr")
                        nc.tensor.transpose(
                            ps, qpt[:, ek, t * 128:(t + 1) * 128], ident)
                        nc.vector.tensor_scalar_mul(
                            qp_b[:, t, ek * 128:(ek + 1) * 128], ps,
                            qm[:, t:t + 1])

                # ---------- context transpose ----------
                ct_r = big.tile([128, DK, CPAD], f32r, tag="ct_r")
                for t in range(CTN):
                    for k in range(DK):
                        ps = ps_tr.tile([128, 128], f32, tag="tr")
                        nc.tensor.transpose(
                            ps, cnat[:, t, k * 128:(k + 1) * 128], ident)
                        evict(ct_r[:, k, t * 128:(t + 1) * 128], ps)

                # ---------- affinity + exp ----------
                e0 = big.tile([128, CTN, QPAD], bf16, tag="e0")
                nc.gpsimd.memset(e0[:, :, QPE:], E0_PAD)
                for t in range(CTN):
                    ps = ps_mm.tile([128, QPE], f32, tag="mm")
                    for dk in range(DK):
                        nc.tensor.matmul(
                            ps, ct_r[:, dk, t * 128:(t + 1) * 128],
                            qpt_r[:, dk, 0:QPE],
                            start=(dk == 0), stop=(dk == DK - 1))
                    nc.scalar.activation(e0[:, t, 0:QPE], ps, AF.Exp, bias=neg_g)

                # ---------- E0 transpose ----------
                e0t = big.tile([128, QTN, CPAD], bf16, tag="e0t")
                for t in range(CTN):
                    for qt_i in range(QTN):
                        psb = ps_tr.tile([128, 128], bf16, tag="tr")
                        nc.tensor.transpose(
                            psb, e0[:, t, qt_i * 128:(qt_i + 1) * 128], ident_b)
                        nc.vector.tensor_copy(
                            e0t[:, qt_i, t * 128:(t + 1) * 128], psb)

                # row ranges for streaming outputs (drop sentinel row)
                def crows(t):
                    if t == 0:
                        return 1, 128, 0
                    if t == CTN - 1:
                        return 0, 1, t * 128 - 1
                    return 0, 128, t * 128 - 1

                def qrows(t):
                    if t == 0:
                        return 1, 128, 0
                    if t == QTN - 1:
                        return 0, 1, t * 128 - 1
                    return 0, 128, t * 128 - 1

                # ---------- S_c = (1/Z_c) E_c^T C ----------
                sc_f = big.tile([128, QTN, D], f32, tag="scf")
                sc_b = big.tile([128, QTN, D], bf16, tag="scb")
                rzc = big.tile([128, QTN], f32, tag="rzc")
                for t in range(QTN):
                    ps = ps_mm.tile([128, D], f32, tag="mm")
                    pz = ps_z.tile([128, 1], f32, tag="z")
                    for ck in range(CTN):
                        lhs = e0[:, ck, t * 128:(t + 1) * 128]
                        nc.tensor.matmul(ps, lhs, cmask[:, ck, :],
                                         start=(ck == 0), stop=(ck == CTN - 1))
                        nc.tensor.matmul(pz, lhs, cmb[:, ck:ck + 1],
                                         start=(ck == 0), stop=(ck == CTN - 1))
                    nc.vector.reciprocal(rzc[:, t:t + 1], pz)
                    evict(sc_f[:, t, :], ps, scale=rzc[:, t:t + 1])
                    nc.gpsimd.tensor_scalar_mul(
                        sc_b[:, t, :], sc_f[:, t, :], qm[:, t:t + 1])
                    r0, r1, o0 = qrows(t)
                    nc.sync.dma_start(
                        out=outq[b, o0:o0 + (r1 - r0), D:2 * D],
                        in_=sc_f[r0:r1, t, :])

                # ---------- S_q = (1/Z_q) E_q^T Qp ----------
                sq_f = big.tile([128, CTN, D], f32, tag="ct_r")  # reuse slot
                sq_b = big.tile([128, CTN, D], bf16, tag="sqb")
                rzq = big.tile([128, CTN], f32, tag="rzq")
                for t in range(CTN):
                    ps = ps_mm.tile([128, D], f32, tag="mm")
                    pz = ps_z.tile([128, 1], f32, tag="z")
                    for qk in range(QTN):
                        lhs = e0t[:, qk, t * 128:(t + 1) * 128]
                        nc.tensor.matmul(ps, lhs, qp_b[:, qk, :],
                                         start=(qk == 0), stop=(qk == QTN - 1))
                        nc.tensor.matmul(pz, lhs, qmb[:, qk:qk + 1],
                                         start=(qk == 0), stop=(qk == QTN - 1))
                    nc.vector.reciprocal(rzq[:, t:t + 1], pz)
                    evict(sq_f[:, t, :], ps, scale=rzq[:, t:t + 1])
                    nc.gpsimd.tensor_scalar_mul(
                        sq_b[:, t, :], sq_f[:, t, :], cm[:, t:t + 1])
                    r0, r1, o0 = crows(t)
                    nc.sync.dma_start(
                        out=outc[b, o0:o0 + (r1 - r0), D:2 * D],
                        in_=sq_f[r0:r1, t, :])

                # ---------- CC = (1/Z_q) E_q^T S_c ----------
                for t in range(CTN):
                    ps = ps_mm.tile([128, D], f32, tag="mm")
                    for qk in range(QTN):
                        nc.tensor.matmul(
                            ps, e0t[:, qk, t * 128:(t + 1) * 128],
                            sc_b[:, qk, :],
                            start=(qk == 0), stop=(qk == QTN - 1))
                    st = stage.tile([128, D], f32, tag="st")
                    evict(st, ps, scale=rzq[:, t:t + 1])
                    r0, r1, o0 = crows(t)
                    nc.sync.dma_start(
                        out=outc[b, o0:o0 + (r1 - r0), 0:D], in_=st[r0:r1, :])

                # ---------- CQ = (1/Z_c) E_c^T S_q ----------
                for t in range(QTN):
                    ps = ps_mm.tile([128, D], f32, tag="mm")
                    for ck in range(CTN):
                        nc.tensor.matmul(
                            ps, e0[:, ck, t * 128:(t + 1) * 128],
                            sq_b[:, ck, :],
                            start=(ck == 0), stop=(ck == CTN - 1))
                    st = stage.tile([128, D], f32, tag="st")
                    evict(st, ps, scale=rzc[:, t:t + 1])
                    r0, r1, o0 = qrows(t)
                    nc.sync.dma_start(
                        out=outq[b, o0:o0 + (r1 - r0), 0:D], in_=st[r0:r1, :])

    nc.compile()
    return nc


def _get_nc(repeat=1):
    key = ("nc", repeat)
    if key not in _CACHE:
        _CACHE[key] = _build(repeat)
    return _CACHE[key]


def _prepare_in_maps(context, question, context_padding, question_padding,
                     proj_w, proj_b, sentinel):
    context = np.asarray(context, np.float32)
    question = np.asarray(question, np.float32)
    cpad = np.asarray(context_padding, bool)
    qpad = np.asarray(question_padding, bool)
    proj_wt = np.ascontiguousarray(np.asarray(proj_w, np.float32).T)
    b_pt = np.ascontiguousarray(
        np.asarray(proj_b, np.float32).reshape(DK, 128).T)
    sent = np.ascontiguousarray(np.asarray(sentinel, np.float32))

    # question^T with sentinel column 0 and zero pad columns: [B, D, QPAD]
    q0t = np.zeros((B, D, QPAD), np.float32)
    q0t[:, :, 0] = sent[1]
    q0t[:, :, 1:LQ + 1] = question.transpose(0, 2, 1)

    # 0/1 key-keep masks in [partition, tile] layout; sentinel kept,
    # pad rows (c >= 2049 / q >= 257) dropped.
    cm01 = np.zeros((B, CPAD), np.float32)
    cm01[:, 0] = 1.0
    cm01[:, 1:LC + 1] = 1.0 - cpad.astype(np.float32)
    cm01 = np.ascontiguousarray(cm01.reshape(B, CTN, 128).transpose(0, 2, 1))
    qm01 = np.zeros((B, QPAD), np.float32)
    qm01[:, 0] = 1.0
    qm01[:, 1:LQ + 1] = 1.0 - qpad.astype(np.float32)
    qm01 = np.ascontiguousarray(qm01.reshape(B, QTN, 128).transpose(0, 2, 1))

    in_maps = []
    for i in range(NCORES):
        s = slice(i * BLOC, (i + 1) * BLOC)
        in_maps.append({
            "context": np.ascontiguousarray(context[s]),
            "q0t": np.ascontiguousarray(q0t[s]),
            "sent": sent,
            "proj_wt": proj_wt,
            "b_pt": b_pt,
            "cm01": np.ascontiguousarray(cm01[s]),
            "qm01": np.ascontiguousarray(qm01[s]),
        })
    return in_maps


def _run(inputs, trace=False):
    from concourse.bass_utils import run_bass_kernel_spmd

    nc = _get_nc()
    in_maps = _prepare_in_maps(**inputs)
    res = run_bass_kernel_spmd(nc, in_maps, core_ids=list(range(NCORES)),
                               trace=trace)
    out_c = np.concatenate(
        [np.asarray(res.results[i]["out_c"]) for i in range(NCORES)], axis=0)
    out_q = np.concatenate(
        [np.asarray(res.results[i]["out_q"]) for i in range(NCORES)], axis=0)
    return (out_c.astype(np.float32, copy=False),
            out_q.astype(np.float32, copy=False)), res


def kernel(**inputs):
    outs, _ = _run(inputs, trace=False)
    return outs


# revision 5
# speedup vs baseline: 1.0092x; 1.0092x over previous
"""Coattentive layer kernel for Trainium2, data-parallel over batch on 8 cores.

Per batch (Lc=2048, Lq=256, d=512; sentinel-prepended => c:2049 (pad 2176),
q:257 (pad 384)):

  QpT[e,q]   = tanh(W^T-projection of question^T + b)          (fp32r matmuls)
  aff[c,q]   = C @ Qp^T  via CT (PE-transposed context)        (fp32r matmuls)
  E0[c,q]    = exp(aff - 45) stored bf16; E0T = PE-transpose of E0.
  Masked-softmax normalizers are N=1 ones-column matmuls sharing the value
  matmuls' stationary weights; pad/key masks are folded multiplicatively into
  the bf16 casts of the value-side operands (Cmask, Qp, S_c, S_q).
  S_c  = (1/Z_c) E_c^T C           -> out_q[:, 512:]
  S_q  = (1/Z_q) E_q^T Qp          -> out_c[:, 512:]
  CC   = (1/Z_q) E_q^T S_c         -> out_c[:, :512]
  CQ   = (1/Z_c) E_c^T S_q         -> out_q[:, :512]
"""

import numpy as np

B, LC, LQ, D = 16, 2048, 256, 512
NCORES = 8
BLOC = B // NCORES
CP = LC + 1       # 2049
QP = LQ + 1       # 257
QPE = QP + 1      # 258: fp32r matmuls need an even moving free dim
CTN = 17          # c tiles (2049 -> 2176)
QTN = 3           # q tiles (257 -> 384)
CPAD = CTN * 128  # 2176
QPAD = QTN * 128  # 384
DK = 4            # 512 / 128
NEG_G = -45.0     # global exp shift (affinity range is ~[-84, 86])
E0_PAD = 1e-20    # keeps Z > 0 for padding rows (avoids 1/0 -> NaN)

_CACHE = {}


def _build(repeat=1):
    import contextlib
    import concourse.bacc as bacc
    import concourse.tile as tile
    from concourse import mybir
    from concourse.masks import make_identity

    f32 = mybir.dt.float32
    f32r = mybir.dt.float32r
    bf16 = mybir.dt.bfloat16
    AF = mybir.ActivationFunctionType

    nc = bacc.Bacc("TRN2", target_bir_lowering=False, debug=False,
                   num_devices=NCORES)

    ctx_in = nc.declare_dram_parameter("context", [BLOC, LC, D], f32, False)
    q0t_in = nc.declare_dram_parameter("q0t", [BLOC, D, QPAD], f32, False)
    sent_in = nc.declare_dram_parameter("sent", [2, D], f32, False)
    wt_in = nc.declare_dram_parameter("proj_wt", [D, D], f32, False)
    b_in = nc.declare_dram_parameter("b_pt", [128, DK], f32, False)
    cm_in = nc.declare_dram_parameter("cm01", [BLOC, 128, CTN], f32, False)
    qm_in = nc.declare_dram_parameter("qm01", [BLOC, 128, QTN], f32, False)
    outc = nc.declare_dram_parameter("out_c", [BLOC, LC, 2 * D], f32, True)
    outq = nc.declare_dram_parameter("out_q", [BLOC, LQ, 2 * D], f32, True)

    with tile.TileContext(nc) as tc:
        with (
            tc.tile_pool(name="consts", bufs=1) as consts,
            tc.tile_pool(name="big", bufs=1) as big,
            tc.tile_pool(name="stage", bufs=3) as stage,
            tc.tile_pool(name="ps_mm", bufs=4, space="PSUM") as ps_mm,
            tc.tile_pool(name="ps_z", bufs=2, space="PSUM") as ps_z,
            tc.tile_pool(name="ps_tr", bufs=2, space="PSUM") as ps_tr,
        ):
            ident = consts.tile([128, 128], f32, tag="ident")
            make_identity(nc, ident)
            ident_b = consts.tile([128, 128], bf16, tag="identb")
            nc.vector.tensor_copy(ident_b, ident)
            neg_g = consts.tile([128, 1], f32, tag="negg")
            nc.vector.memset(neg_g, NEG_G)

            wt_f = consts.tile([128, DK, D], f32, tag="wtf")
            nc.sync.dma_start(
                out=wt_f, in_=wt_in[:].rearrange("(k p) e -> p k e", p=128))
            wt_r = consts.tile([128, DK, D], f32r, tag="wtr")
            nc.vector.tensor_copy(wt_r, wt_f)
            b_sb = consts.tile([128, DK], f32, tag="bsb")
            nc.sync.dma_start(out=b_sb, in_=b_in[:])

            # alternating psum-eviction engine (balances ACT vs DVE)
            ev_state = [0]

            def evict(dst, src, scale=None):
                ev_state[0] ^= 1
                if ev_state[0]:
                    nc.scalar.activation(
                        dst, src, AF.Copy,
                        **({"scale": scale} if scale is not None else {}))
                elif scale is not None:
                    nc.vector.tensor_scalar_mul(dst, src, scale)
                else:
                    nc.vector.tensor_copy(dst, src)

            rep_ctx = (tc.For_i(0, repeat, 1,
                                hint_engines=(mybir.EngineType.PE,
                                              mybir.EngineType.Activation,
                                              mybir.EngineType.DVE,
                                              mybir.EngineType.SP))
                       if repeat > 1 else contextlib.nullcontext())
            with rep_ctx:
              for b in range(BLOC):
                # ---------- loads ----------
                cnat = big.tile([128, CTN, D], f32, tag="cnat")
                nc.vector.memset(cnat[:, CTN - 1, :], 0.0)
                nc.sync.dma_start(out=cnat[0:1, 0, :], in_=sent_in[0:1, :])
                nc.sync.dma_start(out=cnat[1:128, 0, :], in_=ctx_in[b, 0:127, :])
                for t in range(1, CTN - 1):
                    nc.sync.dma_start(
                        out=cnat[:, t, :],
                        in_=ctx_in[b, t * 128 - 1:t * 128 + 127, :])
                nc.sync.dma_start(
                    out=cnat[0:1, CTN - 1, :], in_=ctx_in[b, LC - 1:LC, :])

                q0t = big.tile([128, DK, QPAD], f32r, tag="q0t")
                nc.sync.dma_start(
                    out=q0t,
                    in_=q0t_in[b].rearrange("(k p) q -> p k q", p=128).bitcast(f32r))

                cm = big.tile([128, CTN], f32, tag="cm")
                nc.sync.dma_start(out=cm, in_=cm_in[b])
                qm = big.tile([128, QTN], f32, tag="qm")
                nc.sync.dma_start(out=qm, in_=qm_in[b])
                cmb = big.tile([128, CTN], bf16, tag="cmb")
                nc.vector.tensor_copy(cmb, cm)
                qmb = big.tile([128, QTN], bf16, tag="qmb")
                nc.vector.tensor_copy(qmb, qm)

                # ---------- projection: QpT = tanh(W^T q + b) ----------
                qpt = big.tile([128, DK, QPAD], f32, tag="qpt")
                qpt_r = big.tile([128, DK, QPAD], f32r, tag="qptr")
                nc.vector.memset(qpt[:, :, QPE:], 0.0)
                for ek in range(DK):
                    ps = ps_mm.tile([128, QPE], f32, tag="mm")
                    for dk in range(DK):
                        nc.tensor.matmul(
                            ps, wt_r[:, dk, ek * 128:(ek + 1) * 128],
                            q0t[:, dk, 0:QPE],
                            start=(dk == 0), stop=(dk == DK - 1))
                    nc.scalar.activation(
                        qpt[:, ek, 0:QPE], ps, AF.Tanh, bias=b_sb[:, ek:ek + 1])
                    nc.vector.tensor_copy(qpt_r[:, ek, 0:QPE], qpt[:, ek, 0:QPE])

                # ---------- Qp natural (bf16, q-mask folded) ----------
                qp_b = big.tile([128, QTN, D], bf16, tag="qp")
                for t in range(QTN):
                    for ek in range(DK):
                        ps = ps_tr.tile([128, 128], f32, tag="tr")
                        nc.tensor.transpose(
                            ps, qpt[:, ek, t * 128:(t + 1) * 128], ident)
                        nc.vector.tensor_scalar_mul(
                            qp_b[:, t, ek * 128:(ek + 1) * 128], ps,
                            qm[:, t:t + 1])

                # ---------- context transpose ----------
                ct_r = big.tile([128, DK, CPAD], f32r, tag="ct_r")
                for t in range(CTN):
                    for k in range(DK):
                        ps = ps_tr.tile([128, 128], f32, tag="tr")
                        nc.tensor.transpose(
                            ps, cnat[:, t, k * 128:(k + 1) * 128], ident)
                        evict(ct_r[:, k, t * 128:(t + 1) * 128], ps)

                # bf16 masked context for the value matmuls (on Pool engine)
                cmask = big.tile([128, CTN, D], bf16, tag="cmask")
                for t in range(CTN):
                    eng = nc.vector if t % 2 == 0 else nc.scalar
                    if eng is nc.vector:
                        nc.vector.tensor_scalar_mul(
                            cmask[:, t, :], cnat[:, t, :], cm[:, t:t + 1])
                    else:
                        nc.scalar.activation(
                            cmask[:, t, :], cnat[:, t, :], AF.Copy,
                            scale=cm[:, t:t + 1])

                # ---------- affinity + exp ----------
                e0 = big.tile([128, CTN, QPAD], bf16, tag="e0")
                nc.vector.memset(e0[:, :, QPE:], E0_PAD)
                for t in range(CTN):
                    ps = ps_mm.tile([128, QPE], f32, tag="mm")
                    for dk in range(DK):
                        nc.tensor.matmul(
                            ps, ct_r[:, dk, t * 128:(t + 1) * 128],
                            qpt_r[:, dk, 0:QPE],
                            start=(dk == 0), stop=(dk == DK - 1))
                    nc.scalar.activation(e0[:, t, 0:QPE], ps, AF.Exp, bias=neg_g)

                # ---------- E0 transpose ----------
                e0t = big.tile([128, QTN, CPAD], bf16, tag="e0t")
                for t in range(CTN):
                    for qt_i in range(QTN):
                        psb = ps_tr.tile([128, 128], bf16, tag="tr")
                        nc.tensor.transpose(
                            psb, e0[:, t, qt_i * 128:(qt_i + 1) * 128], ident_b)
                        nc.vector.tensor_copy(
                            e0t[:, qt_i, t * 128:(t + 1) * 128], psb)

                # row ranges for streaming outputs (drop sentinel row)
                def crows(t):
                    if t == 0:
                        return 1, 128, 0
                    if t == CTN - 1:
                        return 0, 1, t * 128 - 1
                    return 0, 128, t * 128 - 1

                def qrows(t):
                    if t == 0:
                        return 1, 128, 0
                    if t == QTN - 1:
                        return 0, 1, t * 128 - 1
                    return 0, 128, t * 128 - 1

                # ---------- S_c = (1/Z_c) E_c^T C ----------
                sc_f = big.tile([128, QTN, D], f32, tag="scf")
                sc_b = big.tile([128, QTN, D], bf16, tag="scb")
                rzc = big.tile([128, QTN], f32, tag="rzc")
                for t in range(QTN):
                    ps = ps_mm.tile([128, D], f32, tag="mm")
                    pz = ps_z.tile([128, 1], f32, tag="z")
                    for ck in range(CTN):
                        lhs = e0[:, ck, t * 128:(t + 1) * 128]
                        nc.tensor.matmul(ps, lhs, cmask[:, ck, :],
                                         start=(ck == 0), stop=(ck == CTN - 1))
                        nc.tensor.matmul(pz, lhs, cmb[:, ck:ck + 1],
                                         start=(ck == 0), stop=(ck == CTN - 1))
                    nc.vector.reciprocal(rzc[:, t:t + 1], pz)
                    evict(sc_f[:, t, :], ps, scale=rzc[:, t:t + 1])
                    nc.vector.tensor_scalar_mul(
                        sc_b[:, t, :], sc_f[:, t, :], qm[:, t:t + 1])
                    r0, r1, o0 = qrows(t)
                    nc.sync.dma_start(
                        out=outq[b, o0:o0 + (r1 - r0), D:2 * D],
                        in_=sc_f[r0:r1, t, :])

                # ---------- S_q = (1/Z_q) E_q^T Qp ----------
                sq_f = big.tile([128, CTN, D], f32, tag="ct_r")  # reuse slot
                sq_b = big.tile([128, CTN, D], bf16, tag="sqb")
                rzq = big.tile([128, CTN], f32, tag="rzq")
                for t in range(CTN):
                    ps = ps_mm.tile([128, D], f32, tag="mm")
                    pz = ps_z.tile([128, 1], f32, tag="z")
                    for qk in range(QTN):
                        lhs = e0t[:, qk, t * 128:(t + 1) * 128]
                        nc.tensor.matmul(ps, lhs, qp_b[:, qk, :],
                                         start=(qk == 0), stop=(qk == QTN - 1))
                        nc.tensor.matmul(pz, lhs, qmb[:, qk:qk + 1],
                                         start=(qk == 0), stop=(qk == QTN - 1))
                    nc.vector.reciprocal(rzq[:, t:t + 1], pz)
                    evict(sq_f[:, t, :], ps, scale=rzq[:, t:t + 1])
                    nc.scalar.activation(
                        sq_b[:, t, :], sq_f[:, t, :], AF.Copy,
                        scale=cm[:, t:t + 1])
                    r0, r1, o0 = crows(t)
                    nc.sync.dma_start(
                        out=outc[b, o0:o0 + (r1 - r0), D:2 * D],
                        in_=sq_f[r0:r1, t, :])

                # ---------- CC = (1/Z_q) E_q^T S_c ----------
                for t in range(CTN):
                    ps = ps_mm.tile([128, D], f32, tag="mm")
                    for qk in range(QTN):
                        nc.tensor.matmul(
                            ps, e0t[:, qk, t * 128:(t + 1) * 128],
                            sc_b[:, qk, :],
                            start=(qk == 0), stop=(qk == QTN - 1))
                    st = stage.tile([128, D], f32, tag="st")
                    evict(st, ps, scale=rzq[:, t:t + 1])
                    r0, r1, o0 = crows(t)
                    nc.sync.dma_start(
                        out=outc[b, o0:o0 + (r1 - r0), 0:D], in_=st[r0:r1, :])

                # ---------- CQ = (1/Z_c) E_c^T S_q ----------
                for t in range(QTN):
                    ps = ps_mm.tile([128, D], f32, tag="mm")
                    for ck in range(CTN):
                        nc.tensor.matmul(
                            ps, e0[:, ck, t * 128:(t + 1) * 128],
                            sq_b[:, ck, :],
                            start=(ck == 0), stop=(ck == CTN - 1))
                    st = stage.tile([128, D], f32, tag="st")
                    evict(st, ps, scale=rzc[:, t:t + 1])
                    r0, r1, o0 = qrows(t)
                    nc.sync.dma_start(
                        out=outq[b, o0:o0 + (r1 - r0), 0:D], in_=st[r0:r1, :])

    nc.compile()
    return nc


def _get_nc(repeat=1):
    key = ("nc", repeat)
    if key not in _CACHE:
        _CACHE[key] = _build(repeat)
    return _CACHE[key]


def _prepare_in_maps(context, question, context_padding, question_padding,
                     proj_w, proj_b, sentinel):
    context = np.asarray(context, np.float32)
    question = np.asarray(question, np.float32)
    cpad = np.asarray(context_padding, bool)
    qpad = np.asarray(question_padding, bool)
    proj_wt = np.ascontiguousarray(np.asarray(proj_w, np.float32).T)
    b_pt = np.ascontiguousarray(
        np.asarray(proj_b, np.float32).reshape(DK, 128).T)
    sent = np.ascontiguousarray(np.asarray(sentinel, np.float32))

    # question^T with sentinel column 0 and zero pad columns: [B, D, QPAD]
    q0t = np.zeros((B, D, QPAD), np.float32)
    q0t[:, :, 0] = sent[1]
    q0t[:, :, 1:LQ + 1] = question.transpose(0, 2, 1)

    # 0/1 key-keep masks in [partition, tile] layout; sentinel kept,
    # pad rows (c >= 2049 / q >= 257) dropped.
    cm01 = np.zeros((B, CPAD), np.float32)
    cm01[:, 0] = 1.0
    cm01[:, 1:LC + 1] = 1.0 - cpad.astype(np.float32)
    cm01 = np.ascontiguousarray(cm01.reshape(B, CTN, 128).transpose(0, 2, 1))
    qm01 = np.zeros((B, QPAD), np.float32)
    qm01[:, 0] = 1.0
    qm01[:, 1:LQ + 1] = 1.0 - qpad.astype(np.float32)
    qm01 = np.ascontiguousarray(qm01.reshape(B, QTN, 128).transpose(0, 2, 1))

    in_maps = []
    for i in range(NCORES):
        s = slice(i * BLOC, (i + 1) * BLOC)
        in_maps.append({
            "context": np.ascontiguousarray(context[s]),
            "q0t": np.ascontiguousarray(q0t[s]),
            "sent": sent,
            "proj_wt": proj_wt,
            "b_pt": b_pt,
            "cm01": np.ascontiguousarray(cm01[s]),
            "qm01": np.ascontiguousarray(qm01[s]),
        })
    return in_maps


def _run(inputs, trace=False):
    from concourse.bass_utils import run_bass_kernel_spmd

    nc = _get_nc()
    in_maps = _prepare_in_maps(**inputs)
    res = run_bass_kernel_spmd(nc, in_maps, core_ids=list(range(NCORES)),
                               trace=trace)
    out_c = np.concatenate(
        [np.asarray(res.results[i]["out_c"]) for i in range(NCORES)], axis=0)
    out_q = np.concatenate(
        [np.asarray(res.results[i]["out_q"]) for i in range(NCORES)], axis=0)
    return (out_c.astype(np.float32, copy=False),
            out_q.astype(np.float32, copy=False)), res


def kernel(**inputs):
    outs, _ = _run(inputs, trace=False)
    return outs
